# revision 2
# baseline (speedup 1.0000x reference)
"""Trainium2 Bass kernel v2 for nn_ExpandFormerV15Complete (moe_routing).

Per token t (vocab id v): y = embed[v] + 0.1 * gelu(embed[v] @ W1[d]) @ W2[d]
if member[v, d] == 1 for some (unique) d, else y = embed[v].

Design vs the v1 baseline (94us):
- Host-side routing: each core's 4096 tokens are permuted so domain-d
  members form contiguous segments (padded to a core-uniform width W[d]
  with nonmember "filler" tokens whose correction is killed by cmask);
  remaining nonmembers trail.  The expert MLP runs once per domain on
  its segment only (~2.6K cols) instead of 8 domains x all 4K tokens.
- bf16 everywhere (table, weights, y) - tolerance is 2e-2.
- The ANT dma_gather ucode (~8ns/idx) is the fundamental bottleneck on
  one DSP pair, but with num_swdge_queues=4 the four 1024-idx gathers
  run their descriptor generation on four DSP pairs in parallel.  Only
  the first ANT instruction on the engine runs synchronously, so a
  16-idx dummy gather absorbs that; the four real gathers dispatch in
  ~100ns each and overlap.
- The ucode library load (~11us) is issued before the Tile preamble so
  it overlaps the input DMAs.
- Parity select (pair-packed rows, idx = v>>1 to fit int16) is done in
  place on the gather tiles; member tiles are PE-transposed (batched
  4-to-a-PSUM-bank, one scalar copy per batch) into hTp; nonmember
  tiles' selected h IS y and DMAs straight out per chunk.
"""

import numpy as np
import ml_dtypes

import concourse.bass as bass
import concourse.bacc as bacc
import concourse.tile as tile
import concourse.mybir as mybir
from concourse import library_config
from concourse.bass_utils import run_bass_kernel_spmd

VOCAB = 50257
BASE = 64
NDOM = 8
HID = 128
B, S = 16, 2048
CORR = 0.1

NCORES = 8
TOK = (B * S) // NCORES          # 4096 tokens per core
PAIR_ROWS = (VOCAB + 1) // 2 + 1  # 25130; extra zero pair-row for dead slots

F32 = mybir.dt.float32
BF16 = mybir.dt.bfloat16
I16 = mybir.dt.int16
I32 = mybir.dt.int32
U8 = mybir.dt.uint8

CHS = [512] * 8                  # 8 gather chunks, queues 1,2,3,0,...
QPAT = [1, 2, 3, 0]              # q0 blocks the engine but its DSP pair still works


def _install_tile_fix():
    """This walrus build rejects Drain instructions with >1 sync wait.
    Tile's exit barrier attaches one wait per DMA-sem lane to its tail
    drain; split them into a chain of single-wait drains."""
    if getattr(tile.TileContext, "_drain_split_installed", False):
        return

    def _patched(self, tick_clock, wait_clock):
        from concourse.vector_clock import ScopedClock

        drain_inst = self.nc.sync.drain()
        wait_clock.add_sem_waits(
            drain_inst.ins, ScopedClock({None: tick_clock.global_clock})
        )
        si = drain_inst.ins.sync_info
        if si is not None and si.on_wait and len(si.on_wait) > 1:
            waits = list(si.on_wait)
            si.on_wait = waits[:1]
            for w in waits[1:]:
                d2 = self.nc.sync.drain()
                si2 = d2.ins.sync_info
                if si2 is None:
                    d2.ins.sync_info = type(si)(on_wait=[w], on_update=[])
                else:
                    si2.on_wait = list(si2.on_wait) + [w]
        self.nc.all_engine_barrier()
        popped = self.nc._tile_sem_poison_stack.pop()
        assert popped is self._sem_poison
        self.nc.clear_and_free_semaphores(list(self.sems.allocated().values()))
        self.nc.all_engine_barrier()

    tile.TileContext._drain_and_barrier = _patched
    tile.TileContext._drain_split_installed = True


def _build_program(meta):
    """meta: M (member cols, mult of 64), Wd (8 domain widths, sum = M)."""
    _install_tile_fix()
    M = meta["M"]
    Wd = meta["Wd"]
    Mup = (M + 127) // 128 * 128
    kS = M // 128                 # first (possibly straddle) nm tile
    NMT = TOK // 128 - kS

    nc = bacc.Bacc(
        "TRN2", target_bir_lowering=False, debug=False, num_swdge_queues=4
    )

    # idx: wrapped int16 pair ids
    idx_in = nc.declare_dram_parameter("idx", [128, TOK // 16], I16, isOutput=False)
    parP_in = nc.declare_dram_parameter("parP", [128, TOK // 128], U8, isOutput=False)
    # w1e with a 65th all-ones row: contraction picks up the -30 filler bias
    w1e_in = nc.declare_dram_parameter("w1e", [65, NDOM * HID], BF16, isOutput=False)
    # per-slot filler bias (-30 on fillers, 0 on members), lands in hTp row 64
    bias_in = nc.declare_dram_parameter("bias", [1, M], BF16, isOutput=False)
    # packed bf16 consts on 128 partitions: w2s [128, 8*64] | idn [128, 128]
    p128_in = nc.declare_dram_parameter("p128", [128, NDOM * BASE + 128], BF16, isOutput=False)
    t2_in = nc.declare_dram_parameter("t2", [PAIR_ROWS, 128], BF16, isOutput=False)
    ym_out = nc.declare_dram_parameter("ym", [64, M], BF16, isOutput=True)
    ynm_out = nc.declare_dram_parameter("ynm", [128, NMT * 64], BF16, isOutput=True)

    # ucode library DMA starts before the Tile preamble so it overlaps
    # the input loads (~11us; the first ANT instruction waits for it).
    nc.gpsimd.load_library(library_config.mlp)

    with tile.TileContext(nc) as tc:
        with (
            tc.tile_pool(name="const", bufs=1) as cpool,
            tc.tile_pool(name="gP", bufs=len(CHS) + 1) as gPpool,
            tc.tile_pool(name="big", bufs=1) as bigpool,
            tc.tile_pool(name="gel", bufs=3) as gelpool,
            tc.tile_pool(name="ps_tr", bufs=2, space="PSUM") as ps_tr,
            tc.tile_pool(name="ps_a", bufs=2, space="PSUM") as ps_a,
            tc.tile_pool(name="ps_c", bufs=2, space="PSUM") as ps_c,
        ):
            # ---- inputs to SBUF (spread across engine DMA queues) ----
            idx16 = cpool.tile([128, TOK // 16], I16)
            nc.sync.dma_start(out=idx16[:, :], in_=idx_in[:, :])
            parP = cpool.tile([128, TOK // 128], U8, tag="parP")
            nc.scalar.dma_start(out=parP[:, :], in_=parP_in[:, :])
            w1e = cpool.tile([65, NDOM * HID], BF16, tag="w1e")
            nc.scalar.dma_start(out=w1e[:, :], in_=w1e_in[:, :])
            p128 = cpool.tile([128, NDOM * BASE + 128], BF16, tag="p128")
            nc.sync.dma_start(out=p128[:, :], in_=p128_in[:, :])
            w2s = p128[:, 0 : NDOM * BASE]
            idn = p128[:, NDOM * BASE : NDOM * BASE + 128]

            # hTp rows 0:64 = transposed h; row 64 = filler bias (DMA'd)
            hTp = bigpool.tile([65, Mup], BF16, tag="hTp")
            nc.sync.dma_start(out=hTp[64:65, 0:M], in_=bias_in[:, :])
            ym = bigpool.tile([64, M], BF16, tag="ym")

            # preload the Gelu activation table off the critical path
            gel_dum = cpool.tile([128, 16], BF16, tag="gdum2")
            nc.vector.memset(gel_dum[:, :], 0.0)
            nc.scalar.activation(
                gel_dum[:, :], gel_dum[:, :], mybir.ActivationFunctionType.Gelu
            )


            # ---- gathers: queues 1-3 only ----
            # a queue-0 ANT instruction blocks the engine for its whole
            # ucode (pair 0 is both worker and responder); on queues 1-3
            # pair 0 idle-responds immediately, so the three gathers
            # dispatch back-to-back and their descriptor generation runs
            # on three DSP pairs concurrently.
            gPs = []
            off = 0
            for ci, ch in enumerate(CHS):
                g = gPpool.tile([128, ch], BF16, tag=f"gP{ci}")
                nc.gpsimd.dma_gather(
                    out_ap=g[:, :].rearrange("p (j e) -> p j e", e=128),
                    in_ap=t2_in[:, :],
                    idxs_ap=idx16[:, off // 16 : (off + ch) // 16],
                    num_idxs=ch,
                    num_idxs_reg=ch,
                    elem_size=128,
                    single_packet=False,
                    queue_num=QPAT[ci % 4],
                )
                gPs.append((g, off, ch))
                off += ch

            # ---- selects (in place) + nm output + member transposes ----
            for g, off, ch in gPs:
                nj = ch // 128
                g3 = g[:, :].rearrange("p (j e) -> p j e", e=128)
                parb = parP[:, off // 128 : off // 128 + nj, None].to_broadcast(
                    [128, nj, 64]
                )
                nc.vector.copy_predicated(g3[:, :, 0:64], parb, g3[:, :, 64:128])

                # member tiles: PE transposes, 4 per PSUM bank, 1 copy per bank
                j = 0
                while j < nj:
                    base = off + j * 128
                    if base >= M:
                        break
                    nbatch = min(4, nj - j, (Mup - base) // 128)
                    pst = ps_tr.tile([64, 512], BF16)
                    for k in range(nbatch):
                        nc.tensor.matmul(
                            pst[:, k * 128 : (k + 1) * 128],
                            lhsT=g[:, (j + k) * 128 : (j + k) * 128 + 64],
                            rhs=idn[:, :],
                            is_transpose=True,
                            start=True,
                            stop=True,
                        )
                    if (base // 512) % 2 == 0:
                        nc.scalar.copy(
                            out=hTp[0:64, base : base + nbatch * 128],
                            in_=pst[:, 0 : nbatch * 128],
                        )
                    else:
                        nc.vector.tensor_copy(
                            out=hTp[0:64, base : base + nbatch * 128],
                            in_=pst[:, 0 : nbatch * 128],
                        )
                    j += nbatch

                # nm tail of this chunk (incl. straddle tile) -> y out
                jnm = max(0, (M - off) // 128)  # first nm/straddle tile idx
                if jnm < nj:
                    col = (off // 128 + jnm - kS) * 64
                    n = nj - jnm
                    nc.sync.dma_start(
                        out=ynm_out[:, col : col + n * 64],
                        in_=g3[:, jnm:nj, 0:64],
                    )

            # ---- expert MLP per domain ----
            # contraction row 64 carries the filler bias: A = h@W1 - 30 on
            # filler slots, so gelu gives exactly 0 and no cmask is needed.
            borders = []
            o = 0
            for ch in CHS:
                o += ch
                borders.append(o)

            def _pieces(lo, hi):
                cuts = sorted({lo, hi} | {b for b in borders if lo < b < hi})
                for a, b in zip(cuts, cuts[1:]):
                    for c0 in range(a, b, 512):
                        yield c0, min(512, b - c0)

            b0 = 0
            for d in range(NDOM):
                w = Wd[d]
                for lo, cw in _pieces(b0, b0 + w):
                    psa = ps_a.tile([128, 512], F32)
                    nc.tensor.matmul(
                        psa[:, 0:cw],
                        lhsT=w1e[:, d * HID : (d + 1) * HID],
                        rhs=hTp[:, lo : lo + cw],
                        start=True,
                        stop=True,
                    )
                    G = gelpool.tile([128, 512], BF16, tag="G")
                    nc.scalar.activation(
                        G[:, 0:cw], psa[:, 0:cw], mybir.ActivationFunctionType.Gelu
                    )
                    psc = ps_c.tile([64, 512], F32)
                    nc.tensor.matmul(
                        psc[:, 0:cw],
                        lhsT=w2s[:, d * BASE : (d + 1) * BASE],
                        rhs=G[:, 0:cw],
                        start=True,
                        stop=True,
                    )
                    # ym = h + corr (filler corr is exactly 0)
                    nc.vector.tensor_tensor(
                        out=ym[:, lo : lo + cw],
                        in0=psc[:, 0:cw],
                        in1=hTp[0:64, lo : lo + cw],
                        op=mybir.AluOpType.add,
                    )
                b0 += w
            nc.sync.dma_start(out=ym_out[:, :], in_=ym[:, :])

    nc.compile()
    return nc


_CACHED = {}


def _wrap_idx(pid):
    """[n] -> [128, n//16] wrapped int16 (16-partition wrap, replicated x8)."""
    n = len(pid)
    return np.tile(pid.reshape(n // 16, 16).T, (8, 1)).astype(np.int16)


def prepare(x, embed, W1, W2, member):
    x = np.asarray(x).astype(np.int64).reshape(B * S)
    embed = np.asarray(embed, dtype=np.float32)
    W1 = np.asarray(W1, dtype=np.float32)
    W2 = np.asarray(W2, dtype=np.float32)
    member = np.asarray(member, dtype=np.float32)

    mem = member > 0.5
    md_vocab = np.where(mem.any(1), mem.argmax(1), -1)  # [VOCAB]

    doms = []
    counts = np.zeros((NCORES, NDOM), np.int64)
    for c in range(NCORES):
        dc = md_vocab[x[c * TOK : (c + 1) * TOK]]
        doms.append(dc)
        for d in range(NDOM):
            counts[c, d] = int((dc == d).sum())

    Wd = ((counts.max(0) + 63) // 64 * 64).astype(int)
    Wd = np.maximum(Wd, 64)
    M = int(Wd.sum())
    if M > TOK - 128:
        raise RuntimeError("routing infeasible for this input")

    # pair-packed bf16 table (extra zero row at PAIR_ROWS-1)
    t2 = np.zeros((PAIR_ROWS * 2, 64), np.float32)
    t2[:VOCAB] = embed
    t2 = t2.reshape(PAIR_ROWS, 128).astype(ml_dtypes.bfloat16)

    w1e = np.zeros((65, NDOM * HID), np.float32)
    w2s = np.zeros((HID, NDOM * BASE), np.float32)
    for d in range(NDOM):
        w1e[:BASE, d * HID : (d + 1) * HID] = W1[d]
        w1e[BASE, d * HID : (d + 1) * HID] = 1.0   # bias row pickup
        w2s[:, d * BASE : (d + 1) * BASE] = W2[d] * CORR
    w1e = w1e.astype(ml_dtypes.bfloat16)
    w2s = w2s.astype(ml_dtypes.bfloat16)
    idn = np.eye(128, dtype=ml_dtypes.bfloat16)
    p128 = np.hstack([w2s, idn])

    in_maps = []
    perms = []
    for c in range(NCORES):
        xc = x[c * TOK : (c + 1) * TOK].astype(np.int64)
        dc = doms[c]
        fill = list(np.where(dc < 0)[0])
        order = []
        cmask = np.zeros(M, np.float32)
        b0 = 0
        for d in range(NDOM):
            td = list(np.where(dc == d)[0])
            order += td
            cmask[b0 : b0 + len(td)] = 1.0
            pads = int(Wd[d]) - len(td)
            order += fill[:pads]
            fill = fill[pads:]
            b0 += int(Wd[d])
        order += fill
        slots = np.asarray(order, np.int64)          # slot -> core-local token
        assert len(slots) == TOK
        perms.append(slots)

        v = xc[slots]
        pid = (v >> 1).astype(np.int16)
        par = (v & 1).astype(np.uint8)

        chunks = []
        o = 0
        for ch in CHS:
            chunks.append(_wrap_idx(pid[o : o + ch]))
            o += ch
        idx = np.hstack(chunks)
        parP = par.reshape(-1, 128).T.copy()
        bias = (-30.0 * (1.0 - cmask)).astype(ml_dtypes.bfloat16)[None, :]

        in_maps.append(
            {"idx": idx, "parP": parP, "w1e": w1e, "bias": bias,
             "p128": p128, "t2": t2}
        )

    meta = {"M": M, "Wd": [int(w) for w in Wd]}
    return in_maps, perms, meta


def kernel(x, embed, W1, W2, member):
    in_maps, perms, meta = prepare(x, embed, W1, W2, member)
    key = (meta["M"], tuple(meta["Wd"]))
    if key not in _CACHED:
        _CACHED[key] = _build_program(meta)
    nc = _CACHED[key]

    res = run_bass_kernel_spmd(nc, in_maps, core_ids=list(range(NCORES)))

    M = meta["M"]
    kS = M // 128
    out = np.empty((B * S, BASE), np.float32)
    for c in range(NCORES):
        ym = np.asarray(res.results[c]["ym"], dtype=np.float32)      # [64, M]
        ynm = np.asarray(res.results[c]["ynm"], dtype=np.float32)    # [128, nmt*64]
        slots = perms[c]
        y_slots = np.empty((TOK, BASE), np.float32)
        nmt = ynm.shape[1] // 64
        ynm3 = ynm.reshape(128, nmt, 64).transpose(1, 0, 2).reshape(nmt * 128, 64)
        y_slots[kS * 128 :] = ynm3[: TOK - kS * 128]
        # member region last: straddle slots [kS*128, M) must come from ym
        y_slots[:M] = ym.T
        out[c * TOK + slots] = y_slots
    return out.reshape(B, S, BASE)


# revision 3
# speedup vs baseline: 1.1310x; 1.1310x over previous
"""Trainium2 Bass kernel v2 for nn_ExpandFormerV15Complete (moe_routing).

Per token t (vocab id v): y = embed[v] + 0.1 * gelu(embed[v] @ W1[d]) @ W2[d]
if member[v, d] == 1 for some (unique) d, else y = embed[v].

Design vs the v1 baseline (94us):
- Host-side routing: each core's 4096 tokens are permuted so domain-d
  members form contiguous segments (padded to a core-uniform width W[d]
  with nonmember "filler" tokens whose correction is killed by cmask);
  remaining nonmembers trail.  The expert MLP runs once per domain on
  its segment only (~2.6K cols) instead of 8 domains x all 4K tokens.
- bf16 everywhere (table, weights, y) - tolerance is 2e-2.
- The ANT dma_gather ucode (~8-9ns/idx) is the fundamental per-core
  bottleneck, but it runs on ONE DSP pair selected by queue_num; with
  num_swdge_queues=4 and chunks round-robined over queues [1,2,3,0],
  descriptor generation runs on all four DSP pairs concurrently
  (queue-0 instructions block the engine for their whole ucode - pair 0
  is both worker and responder - but nothing else needs gpsimd then;
  queue 1-3 instructions dispatch in ~100ns and run async).  8 chunks
  of 512 keep the gen->trigger->DMA pipeline fine-grained so data
  streams to the consumers every ~1.5us.
- The ucode library load (~10us) is issued before the Tile preamble so
  it overlaps the input DMAs; it is the startup floor (~17us).
- Parity select (pair-packed rows, idx = v>>1 to fit int16) is done in
  place on the gather tiles; member tiles are PE-transposed (batched
  4-to-a-PSUM-bank, one scalar copy per batch) into hTp; nonmember
  tiles' selected h IS y and DMAs straight out per chunk.
"""

import numpy as np
import ml_dtypes

import concourse.bass as bass
import concourse.bacc as bacc
import concourse.tile as tile
import concourse.mybir as mybir
from concourse import library_config
from concourse.bass_utils import run_bass_kernel_spmd

VOCAB = 50257
BASE = 64
NDOM = 8
HID = 128
B, S = 16, 2048
CORR = 0.1

NCORES = 8
TOK = (B * S) // NCORES          # 4096 tokens per core
PAIR_ROWS = (VOCAB + 1) // 2 + 1  # 25130; extra zero pair-row for dead slots

F32 = mybir.dt.float32
BF16 = mybir.dt.bfloat16
I16 = mybir.dt.int16
I32 = mybir.dt.int32
U8 = mybir.dt.uint8

CHS = [512] * 8                  # 8 gather chunks, queues 1,2,3,0,...
QPAT = [1, 2, 3, 0]              # q0 blocks the engine but its DSP pair still works


def _install_tile_fix():
    """This walrus build rejects Drain instructions with >1 sync wait.
    Tile's exit barrier attaches one wait per DMA-sem lane to its tail
    drain; split them into a chain of single-wait drains."""
    if getattr(tile.TileContext, "_drain_split_installed", False):
        return

    def _patched(self, tick_clock, wait_clock):
        from concourse.vector_clock import ScopedClock

        drain_inst = self.nc.sync.drain()
        wait_clock.add_sem_waits(
            drain_inst.ins, ScopedClock({None: tick_clock.global_clock})
        )
        si = drain_inst.ins.sync_info
        if si is not None and si.on_wait and len(si.on_wait) > 1:
            waits = list(si.on_wait)
            si.on_wait = waits[:1]
            for w in waits[1:]:
                d2 = self.nc.sync.drain()
                si2 = d2.ins.sync_info
                if si2 is None:
                    d2.ins.sync_info = type(si)(on_wait=[w], on_update=[])
                else:
                    si2.on_wait = list(si2.on_wait) + [w]
        self.nc.all_engine_barrier()
        popped = self.nc._tile_sem_poison_stack.pop()
        assert popped is self._sem_poison
        self.nc.clear_and_free_semaphores(list(self.sems.allocated().values()))
        self.nc.all_engine_barrier()

    tile.TileContext._drain_and_barrier = _patched
    tile.TileContext._drain_split_installed = True


def _build_program(meta):
    """meta: M (member cols, mult of 64), Wd (8 domain widths, sum = M)."""
    _install_tile_fix()
    M = meta["M"]
    Wd = meta["Wd"]
    Mup = (M + 127) // 128 * 128
    kS = M // 128                 # first (possibly straddle) nm tile
    NMT = TOK // 128 - kS

    nc = bacc.Bacc(
        "TRN2", target_bir_lowering=False, debug=False, num_swdge_queues=4
    )

    # idx: wrapped int16 pair ids
    idx_in = nc.declare_dram_parameter("idx", [128, TOK // 16], I16, isOutput=False)
    parP_in = nc.declare_dram_parameter("parP", [128, TOK // 128], U8, isOutput=False)
    # w1e with a 65th all-ones row: contraction picks up the -30 filler bias
    w1e_in = nc.declare_dram_parameter("w1e", [65, NDOM * HID], BF16, isOutput=False)
    # per-slot filler bias (-30 on fillers, 0 on members), lands in hTp row 64
    bias_in = nc.declare_dram_parameter("bias", [1, M], BF16, isOutput=False)
    # packed bf16 consts on 128 partitions: w2s [128, 8*64] | idn [128, 128]
    p128_in = nc.declare_dram_parameter("p128", [128, NDOM * BASE + 128], BF16, isOutput=False)
    t2_in = nc.declare_dram_parameter("t2", [PAIR_ROWS, 128], BF16, isOutput=False)
    ym_out = nc.declare_dram_parameter("ym", [64, M], BF16, isOutput=True)
    ynm_out = nc.declare_dram_parameter("ynm", [128, NMT * 64], BF16, isOutput=True)

    # ucode library DMA starts before the Tile preamble so it overlaps
    # the input loads (~11us; the first ANT instruction waits for it).
    nc.gpsimd.load_library(library_config.mlp)

    with tile.TileContext(nc) as tc:
        with (
            tc.tile_pool(name="const", bufs=1) as cpool,
            tc.tile_pool(name="gP", bufs=len(CHS) + 1) as gPpool,
            tc.tile_pool(name="big", bufs=1) as bigpool,
            tc.tile_pool(name="gel", bufs=3) as gelpool,
            tc.tile_pool(name="ps_tr", bufs=2, space="PSUM") as ps_tr,
            tc.tile_pool(name="ps_a", bufs=2, space="PSUM") as ps_a,
            tc.tile_pool(name="ps_c", bufs=2, space="PSUM") as ps_c,
        ):
            # ---- inputs to SBUF (spread across engine DMA queues) ----
            idx16 = cpool.tile([128, TOK // 16], I16)
            nc.sync.dma_start(out=idx16[:, :], in_=idx_in[:, :])
            parP = cpool.tile([128, TOK // 128], U8, tag="parP")
            nc.scalar.dma_start(out=parP[:, :], in_=parP_in[:, :])
            w1e = cpool.tile([65, NDOM * HID], BF16, tag="w1e")
            nc.scalar.dma_start(out=w1e[:, :], in_=w1e_in[:, :])
            p128 = cpool.tile([128, NDOM * BASE + 128], BF16, tag="p128")
            nc.sync.dma_start(out=p128[:, :], in_=p128_in[:, :])
            w2s = p128[:, 0 : NDOM * BASE]
            idn = p128[:, NDOM * BASE : NDOM * BASE + 128]

            # hTp rows 0:64 = transposed h; row 64 = filler bias (DMA'd)
            hTp = bigpool.tile([65, Mup], BF16, tag="hTp")
            nc.sync.dma_start(out=hTp[64:65, 0:M], in_=bias_in[:, :])
            ym = bigpool.tile([64, M], BF16, tag="ym")

            # preload the Gelu activation table off the critical path
            gel_dum = cpool.tile([128, 16], BF16, tag="gdum2")
            nc.vector.memset(gel_dum[:, :], 0.0)
            nc.scalar.activation(
                gel_dum[:, :], gel_dum[:, :], mybir.ActivationFunctionType.Gelu
            )


            # ---- gathers: queues 1-3 only ----
            # a queue-0 ANT instruction blocks the engine for its whole
            # ucode (pair 0 is both worker and responder); on queues 1-3
            # pair 0 idle-responds immediately, so the three gathers
            # dispatch back-to-back and their descriptor generation runs
            # on three DSP pairs concurrently.
            gPs = []
            off = 0
            for ci, ch in enumerate(CHS):
                g = gPpool.tile([128, ch], BF16, tag=f"gP{ci}")
                nc.gpsimd.dma_gather(
                    out_ap=g[:, :].rearrange("p (j e) -> p j e", e=128),
                    in_ap=t2_in[:, :],
                    idxs_ap=idx16[:, off // 16 : (off + ch) // 16],
                    num_idxs=ch,
                    num_idxs_reg=ch,
                    elem_size=128,
                    single_packet=False,
                    queue_num=QPAT[ci % 4],
                )
                gPs.append((g, off, ch))
                off += ch

            # ---- selects (in place) + nm output + member transposes ----
            for g, off, ch in gPs:
                nj = ch // 128
                g3 = g[:, :].rearrange("p (j e) -> p j e", e=128)
                parb = parP[:, off // 128 : off // 128 + nj, None].to_broadcast(
                    [128, nj, 64]
                )
                nc.vector.copy_predicated(g3[:, :, 0:64], parb, g3[:, :, 64:128])

                # member tiles: PE transposes, 4 per PSUM bank, 1 copy per bank
                j = 0
                while j < nj:
                    base = off + j * 128
                    if base >= M:
                        break
                    nbatch = min(4, nj - j, (Mup - base) // 128)
                    pst = ps_tr.tile([64, 512], BF16)
                    for k in range(nbatch):
                        nc.tensor.matmul(
                            pst[:, k * 128 : (k + 1) * 128],
                            lhsT=g[:, (j + k) * 128 : (j + k) * 128 + 64],
                            rhs=idn[:, :],
                            is_transpose=True,
                            start=True,
                            stop=True,
                        )
                    if (base // 512) % 2 == 0:
                        nc.scalar.copy(
                            out=hTp[0:64, base : base + nbatch * 128],
                            in_=pst[:, 0 : nbatch * 128],
                        )
                    else:
                        nc.vector.tensor_copy(
                            out=hTp[0:64, base : base + nbatch * 128],
                            in_=pst[:, 0 : nbatch * 128],
                        )
                    j += nbatch

                # nm tail of this chunk (incl. straddle tile) -> y out
                jnm = max(0, (M - off) // 128)  # first nm/straddle tile idx
                if jnm < nj:
                    col = (off // 128 + jnm - kS) * 64
                    n = nj - jnm
                    nc.sync.dma_start(
                        out=ynm_out[:, col : col + n * 64],
                        in_=g3[:, jnm:nj, 0:64],
                    )

            # ---- expert MLP per domain ----
            # contraction row 64 carries the filler bias: A = h@W1 - 30 on
            # filler slots, so gelu gives exactly 0 and no cmask is needed.
            borders = []
            o = 0
            for ch in CHS:
                o += ch
                borders.append(o)

            def _pieces(lo, hi):
                cuts = sorted({lo, hi} | {b for b in borders if lo < b < hi})
                for a, b in zip(cuts, cuts[1:]):
                    for c0 in range(a, b, 512):
                        yield c0, min(512, b - c0)

            b0 = 0
            for d in range(NDOM):
                w = Wd[d]
                for lo, cw in _pieces(b0, b0 + w):
                    psa = ps_a.tile([128, 512], F32)
                    nc.tensor.matmul(
                        psa[:, 0:cw],
                        lhsT=w1e[:, d * HID : (d + 1) * HID],
                        rhs=hTp[:, lo : lo + cw],
                        start=True,
                        stop=True,
                    )
                    G = gelpool.tile([128, 512], BF16, tag="G")
                    nc.scalar.activation(
                        G[:, 0:cw], psa[:, 0:cw], mybir.ActivationFunctionType.Gelu
                    )
                    psc = ps_c.tile([64, 512], F32)
                    nc.tensor.matmul(
                        psc[:, 0:cw],
                        lhsT=w2s[:, d * BASE : (d + 1) * BASE],
                        rhs=G[:, 0:cw],
                        start=True,
                        stop=True,
                    )
                    # ym = h + corr (filler corr is exactly 0)
                    nc.vector.tensor_tensor(
                        out=ym[:, lo : lo + cw],
                        in0=psc[:, 0:cw],
                        in1=hTp[0:64, lo : lo + cw],
                        op=mybir.AluOpType.add,
                    )
                b0 += w
            nc.sync.dma_start(out=ym_out[:, :], in_=ym[:, :])

    nc.compile()
    return nc


_CACHED = {}


def _wrap_idx(pid):
    """[n] -> [128, n//16] wrapped int16 (16-partition wrap, replicated x8)."""
    n = len(pid)
    return np.tile(pid.reshape(n // 16, 16).T, (8, 1)).astype(np.int16)


def prepare(x, embed, W1, W2, member):
    x = np.asarray(x).astype(np.int64).reshape(B * S)
    embed = np.asarray(embed, dtype=np.float32)
    W1 = np.asarray(W1, dtype=np.float32)
    W2 = np.asarray(W2, dtype=np.float32)
    member = np.asarray(member, dtype=np.float32)

    mem = member > 0.5
    md_vocab = np.where(mem.any(1), mem.argmax(1), -1)  # [VOCAB]

    doms = []
    counts = np.zeros((NCORES, NDOM), np.int64)
    for c in range(NCORES):
        dc = md_vocab[x[c * TOK : (c + 1) * TOK]]
        doms.append(dc)
        for d in range(NDOM):
            counts[c, d] = int((dc == d).sum())

    Wd = ((counts.max(0) + 63) // 64 * 64).astype(int)
    Wd = np.maximum(Wd, 64)
    M = int(Wd.sum())
    if M > TOK - 128:
        raise RuntimeError("routing infeasible for this input")

    # pair-packed bf16 table (extra zero row at PAIR_ROWS-1)
    t2 = np.zeros((PAIR_ROWS * 2, 64), np.float32)
    t2[:VOCAB] = embed
    t2 = t2.reshape(PAIR_ROWS, 128).astype(ml_dtypes.bfloat16)

    w1e = np.zeros((65, NDOM * HID), np.float32)
    w2s = np.zeros((HID, NDOM * BASE), np.float32)
    for d in range(NDOM):
        w1e[:BASE, d * HID : (d + 1) * HID] = W1[d]
        w1e[BASE, d * HID : (d + 1) * HID] = 1.0   # bias row pickup
        w2s[:, d * BASE : (d + 1) * BASE] = W2[d] * CORR
    w1e = w1e.astype(ml_dtypes.bfloat16)
    w2s = w2s.astype(ml_dtypes.bfloat16)
    idn = np.eye(128, dtype=ml_dtypes.bfloat16)
    p128 = np.hstack([w2s, idn])

    in_maps = []
    perms = []
    for c in range(NCORES):
        xc = x[c * TOK : (c + 1) * TOK].astype(np.int64)
        dc = doms[c]
        fill = list(np.where(dc < 0)[0])
        order = []
        cmask = np.zeros(M, np.float32)
        b0 = 0
        for d in range(NDOM):
            td = list(np.where(dc == d)[0])
            order += td
            cmask[b0 : b0 + len(td)] = 1.0
            pads = int(Wd[d]) - len(td)
            order += fill[:pads]
            fill = fill[pads:]
            b0 += int(Wd[d])
        order += fill
        slots = np.asarray(order, np.int64)          # slot -> core-local token
        assert len(slots) == TOK
        perms.append(slots)

        v = xc[slots]
        pid = (v >> 1).astype(np.int16)
        par = (v & 1).astype(np.uint8)

        chunks = []
        o = 0
        for ch in CHS:
            chunks.append(_wrap_idx(pid[o : o + ch]))
            o += ch
        idx = np.hstack(chunks)
        parP = par.reshape(-1, 128).T.copy()
        bias = (-30.0 * (1.0 - cmask)).astype(ml_dtypes.bfloat16)[None, :]

        in_maps.append(
            {"idx": idx, "parP": parP, "w1e": w1e, "bias": bias,
             "p128": p128, "t2": t2}
        )

    meta = {"M": M, "Wd": [int(w) for w in Wd]}
    return in_maps, perms, meta


def kernel(x, embed, W1, W2, member):
    in_maps, perms, meta = prepare(x, embed, W1, W2, member)
    key = (meta["M"], tuple(meta["Wd"]))
    if key not in _CACHED:
        _CACHED[key] = _build_program(meta)
    nc = _CACHED[key]

    res = run_bass_kernel_spmd(nc, in_maps, core_ids=list(range(NCORES)))

    M = meta["M"]
    kS = M // 128
    out = np.empty((B * S, BASE), np.float32)
    for c in range(NCORES):
        ym = np.asarray(res.results[c]["ym"], dtype=np.float32)      # [64, M]
        ynm = np.asarray(res.results[c]["ynm"], dtype=np.float32)    # [128, nmt*64]
        slots = perms[c]
        y_slots = np.empty((TOK, BASE), np.float32)
        nmt = ynm.shape[1] // 64
        ynm3 = ynm.reshape(128, nmt, 64).transpose(1, 0, 2).reshape(nmt * 128, 64)
        y_slots[kS * 128 :] = ynm3[: TOK - kS * 128]
        # member region last: straddle slots [kS*128, M) must come from ym
        y_slots[:M] = ym.T
        out[c * TOK + slots] = y_slots
    return out.reshape(B, S, BASE)


# revision 4
# speedup vs baseline: 1.1452x; 1.0126x over previous
"""Trainium2 Bass kernel v2 for nn_ExpandFormerV15Complete (moe_routing).

Per token t (vocab id v): y = embed[v] + 0.1 * gelu(embed[v] @ W1[d]) @ W2[d]
if member[v, d] == 1 for some (unique) d, else y = embed[v].

Design vs the v1 baseline (94us):
- Host-side routing: each core's 4096 tokens are permuted so domain-d
  members form contiguous segments (padded to a core-uniform width W[d]
  with nonmember "filler" tokens whose correction is killed by cmask);
  remaining nonmembers trail.  The expert MLP runs once per domain on
  its segment only (~2.6K cols) instead of 8 domains x all 4K tokens.
- bf16 everywhere (table, weights, y) - tolerance is 2e-2.
- The ANT dma_gather ucode (~8-9ns/idx) is the fundamental per-core
  bottleneck, but it runs on ONE DSP pair selected by queue_num; with
  num_swdge_queues=4 and chunks round-robined over queues [1,2,3,0],
  descriptor generation runs on all four DSP pairs concurrently
  (queue-0 instructions block the engine for their whole ucode - pair 0
  is both worker and responder - but nothing else needs gpsimd then;
  queue 1-3 instructions dispatch in ~100ns and run async).  8 chunks
  of 512 keep the gen->trigger->DMA pipeline fine-grained so data
  streams to the consumers every ~1.5us.
- The ucode library load (~10us) is issued before the Tile preamble so
  it overlaps the input DMAs; it is the startup floor (~17us).
- Parity select (pair-packed rows, idx = v>>1 to fit int16) is done in
  place on the gather tiles; member tiles are PE-transposed (batched
  4-to-a-PSUM-bank, one scalar copy per batch) into hTp; nonmember
  tiles' selected h IS y and DMAs straight out per chunk.
"""

import numpy as np
import ml_dtypes

import concourse.bass as bass
import concourse.bacc as bacc
import concourse.tile as tile
import concourse.mybir as mybir
from concourse import library_config
from concourse.bass_utils import run_bass_kernel_spmd

VOCAB = 50257
BASE = 64
NDOM = 8
HID = 128
B, S = 16, 2048
CORR = 0.1

NCORES = 8
TOK = (B * S) // NCORES          # 4096 tokens per core
PAIR_ROWS = (VOCAB + 1) // 2 + 1  # 25130; extra zero pair-row for dead slots

F32 = mybir.dt.float32
BF16 = mybir.dt.bfloat16
I16 = mybir.dt.int16
I32 = mybir.dt.int32
U8 = mybir.dt.uint8

CHS = [512] * 8                  # 8 gather chunks, queues 1,2,3,0,...
QPAT = [1, 2, 3, 0]              # q0 blocks the engine but its DSP pair still works


def _install_tile_fix():
    """This walrus build rejects Drain instructions with >1 sync wait.
    Tile's exit barrier attaches one wait per DMA-sem lane to its tail
    drain; split them into a chain of single-wait drains."""
    if getattr(tile.TileContext, "_drain_split_installed", False):
        return

    def _patched(self, tick_clock, wait_clock):
        from concourse.vector_clock import ScopedClock

        drain_inst = self.nc.sync.drain()
        wait_clock.add_sem_waits(
            drain_inst.ins, ScopedClock({None: tick_clock.global_clock})
        )
        si = drain_inst.ins.sync_info
        if si is not None and si.on_wait and len(si.on_wait) > 1:
            waits = list(si.on_wait)
            si.on_wait = waits[:1]
            for w in waits[1:]:
                d2 = self.nc.sync.drain()
                si2 = d2.ins.sync_info
                if si2 is None:
                    d2.ins.sync_info = type(si)(on_wait=[w], on_update=[])
                else:
                    si2.on_wait = list(si2.on_wait) + [w]
        self.nc.all_engine_barrier()
        popped = self.nc._tile_sem_poison_stack.pop()
        assert popped is self._sem_poison
        self.nc.clear_and_free_semaphores(list(self.sems.allocated().values()))
        self.nc.all_engine_barrier()

    tile.TileContext._drain_and_barrier = _patched
    tile.TileContext._drain_split_installed = True


def _build_program(meta):
    """meta: M (member cols, mult of 64), Wd (8 domain widths, sum = M)."""
    _install_tile_fix()
    M = meta["M"]
    Wd = meta["Wd"]
    Mup = (M + 127) // 128 * 128
    kS = M // 128                 # first (possibly straddle) nm tile
    NMT = TOK // 128 - kS

    nc = bacc.Bacc(
        "TRN2", target_bir_lowering=False, debug=False, num_swdge_queues=4
    )

    # idx: wrapped int16 pair ids
    idx_in = nc.declare_dram_parameter("idx", [128, TOK // 16], I16, isOutput=False)
    parP_in = nc.declare_dram_parameter("parP", [128, TOK // 128], U8, isOutput=False)
    # w1e with a 65th all-ones row: contraction picks up the -30 filler bias
    w1e_in = nc.declare_dram_parameter("w1e", [65, NDOM * HID], BF16, isOutput=False)
    # per-slot filler bias (-30 on fillers, 0 on members), lands in hTp row 64
    bias_in = nc.declare_dram_parameter("bias", [1, M], BF16, isOutput=False)
    # packed bf16 consts on 128 partitions: w2s [128, 8*64] | idn [128, 128]
    p128_in = nc.declare_dram_parameter("p128", [128, NDOM * BASE + 128], BF16, isOutput=False)
    t2_in = nc.declare_dram_parameter("t2", [PAIR_ROWS, 128], BF16, isOutput=False)
    ym_out = nc.declare_dram_parameter("ym", [64, M], BF16, isOutput=True)
    ynm_out = nc.declare_dram_parameter("ynm", [128, NMT * 64], BF16, isOutput=True)

    # ucode library DMA starts before the Tile preamble so it overlaps
    # the input loads (~11us; the first ANT instruction waits for it).
    nc.gpsimd.load_library(library_config.mlp)

    with tile.TileContext(nc) as tc:
        with (
            tc.tile_pool(name="const", bufs=1) as cpool,
            tc.tile_pool(name="gP", bufs=len(CHS) + 1) as gPpool,
            tc.tile_pool(name="big", bufs=1) as bigpool,
            tc.tile_pool(name="gel", bufs=3) as gelpool,
            tc.tile_pool(name="ps_tr", bufs=2, space="PSUM") as ps_tr,
            tc.tile_pool(name="ps_a", bufs=2, space="PSUM") as ps_a,
            tc.tile_pool(name="ps_c", bufs=2, space="PSUM") as ps_c,
        ):
            # ---- inputs to SBUF (spread across engine DMA queues) ----
            idx16 = cpool.tile([128, TOK // 16], I16)
            nc.sync.dma_start(out=idx16[:, :], in_=idx_in[:, :])
            parP = cpool.tile([128, TOK // 128], U8, tag="parP")
            nc.scalar.dma_start(out=parP[:, :], in_=parP_in[:, :])
            w1e = cpool.tile([65, NDOM * HID], BF16, tag="w1e")
            nc.scalar.dma_start(out=w1e[:, :], in_=w1e_in[:, :])
            p128 = cpool.tile([128, NDOM * BASE + 128], BF16, tag="p128")
            nc.sync.dma_start(out=p128[:, :], in_=p128_in[:, :])
            w2s = p128[:, 0 : NDOM * BASE]
            idn = p128[:, NDOM * BASE : NDOM * BASE + 128]

            # hTp rows 0:64 = transposed h; row 64 = filler bias (DMA'd)
            hTp = bigpool.tile([65, Mup], BF16, tag="hTp")
            nc.sync.dma_start(out=hTp[64:65, 0:M], in_=bias_in[:, :])
            ym = bigpool.tile([64, M], BF16, tag="ym")

            # preload the Gelu activation table off the critical path
            gel_dum = cpool.tile([128, 16], BF16, tag="gdum2")
            nc.vector.memset(gel_dum[:, :], 0.0)
            nc.scalar.activation(
                gel_dum[:, :], gel_dum[:, :], mybir.ActivationFunctionType.Gelu
            )


            # ---- gathers: queues 1-3 only ----
            # a queue-0 ANT instruction blocks the engine for its whole
            # ucode (pair 0 is both worker and responder); on queues 1-3
            # pair 0 idle-responds immediately, so the three gathers
            # dispatch back-to-back and their descriptor generation runs
            # on three DSP pairs concurrently.
            gPs = []
            off = 0
            for ci, ch in enumerate(CHS):
                g = gPpool.tile([128, ch], BF16, tag=f"gP{ci}")
                nc.gpsimd.dma_gather(
                    out_ap=g[:, :].rearrange("p (j e) -> p j e", e=128),
                    in_ap=t2_in[:, :],
                    idxs_ap=idx16[:, off // 16 : (off + ch) // 16],
                    num_idxs=ch,
                    num_idxs_reg=ch,
                    elem_size=128,
                    single_packet=False,
                    queue_num=QPAT[ci % 4],
                )
                gPs.append((g, off, ch))
                off += ch

            # ---- selects (in place) + nm output + member transposes ----
            for g, off, ch in gPs:
                nj = ch // 128
                g3 = g[:, :].rearrange("p (j e) -> p j e", e=128)
                parb = parP[:, off // 128 : off // 128 + nj, None].to_broadcast(
                    [128, nj, 64]
                )
                nc.vector.copy_predicated(g3[:, :, 0:64], parb, g3[:, :, 64:128])

                # member tiles: PE transposes, 4 per PSUM bank, 1 copy per bank
                j = 0
                while j < nj:
                    base = off + j * 128
                    if base >= M:
                        break
                    nbatch = min(4, nj - j, (Mup - base) // 128)
                    pst = ps_tr.tile([64, 512], BF16)
                    for k in range(nbatch):
                        nc.tensor.matmul(
                            pst[:, k * 128 : (k + 1) * 128],
                            lhsT=g[:, (j + k) * 128 : (j + k) * 128 + 64],
                            rhs=idn[:, :],
                            is_transpose=True,
                            start=True,
                            stop=True,
                        )
                    if (base // 512) % 2 == 0:
                        nc.scalar.copy(
                            out=hTp[0:64, base : base + nbatch * 128],
                            in_=pst[:, 0 : nbatch * 128],
                        )
                    else:
                        nc.vector.tensor_copy(
                            out=hTp[0:64, base : base + nbatch * 128],
                            in_=pst[:, 0 : nbatch * 128],
                        )
                    j += nbatch

                # nm tail of this chunk (incl. straddle tile) -> y out
                jnm = max(0, (M - off) // 128)  # first nm/straddle tile idx
                if jnm < nj:
                    col = (off // 128 + jnm - kS) * 64
                    n = nj - jnm
                    nc.sync.dma_start(
                        out=ynm_out[:, col : col + n * 64],
                        in_=g3[:, jnm:nj, 0:64],
                    )

            # ---- expert MLP per domain ----
            # contraction row 64 carries the filler bias: A = h@W1 - 30 on
            # filler slots, so gelu gives exactly 0 and no cmask is needed.
            def _pieces(lo, hi):
                # rhs reads the contiguous hTp buffer, so pieces only need
                # the <=512-column matmul limit (Tile tracks the per-range
                # dependencies on the chunk copies that feed hTp).
                for c0 in range(lo, hi, 512):
                    yield c0, min(512, hi - c0)

            b0 = 0
            for d in range(NDOM):
                w = Wd[d]
                for lo, cw in _pieces(b0, b0 + w):
                    psa = ps_a.tile([128, 512], F32)
                    nc.tensor.matmul(
                        psa[:, 0:cw],
                        lhsT=w1e[:, d * HID : (d + 1) * HID],
                        rhs=hTp[:, lo : lo + cw],
                        start=True,
                        stop=True,
                    )
                    G = gelpool.tile([128, 512], BF16, tag="G")
                    nc.scalar.activation(
                        G[:, 0:cw], psa[:, 0:cw], mybir.ActivationFunctionType.Gelu
                    )
                    psc = ps_c.tile([64, 512], F32)
                    nc.tensor.matmul(
                        psc[:, 0:cw],
                        lhsT=w2s[:, d * BASE : (d + 1) * BASE],
                        rhs=G[:, 0:cw],
                        start=True,
                        stop=True,
                    )
                    # ym = h + corr (filler corr is exactly 0)
                    nc.vector.tensor_tensor(
                        out=ym[:, lo : lo + cw],
                        in0=psc[:, 0:cw],
                        in1=hTp[0:64, lo : lo + cw],
                        op=mybir.AluOpType.add,
                    )
                b0 += w
            nc.sync.dma_start(out=ym_out[:, :], in_=ym[:, :])

    nc.compile()
    return nc


_CACHED = {}


def _wrap_idx(pid):
    """[n] -> [128, n//16] wrapped int16 (16-partition wrap, replicated x8)."""
    n = len(pid)
    return np.tile(pid.reshape(n // 16, 16).T, (8, 1)).astype(np.int16)


def prepare(x, embed, W1, W2, member):
    x = np.asarray(x).astype(np.int64).reshape(B * S)
    embed = np.asarray(embed, dtype=np.float32)
    W1 = np.asarray(W1, dtype=np.float32)
    W2 = np.asarray(W2, dtype=np.float32)
    member = np.asarray(member, dtype=np.float32)

    mem = member > 0.5
    md_vocab = np.where(mem.any(1), mem.argmax(1), -1)  # [VOCAB]

    doms = []
    counts = np.zeros((NCORES, NDOM), np.int64)
    for c in range(NCORES):
        dc = md_vocab[x[c * TOK : (c + 1) * TOK]]
        doms.append(dc)
        for d in range(NDOM):
            counts[c, d] = int((dc == d).sum())

    Wd = ((counts.max(0) + 63) // 64 * 64).astype(int)
    Wd = np.maximum(Wd, 64)
    M = int(Wd.sum())
    if M > TOK - 128:
        raise RuntimeError("routing infeasible for this input")

    # pair-packed bf16 table (extra zero row at PAIR_ROWS-1)
    t2 = np.zeros((PAIR_ROWS * 2, 64), np.float32)
    t2[:VOCAB] = embed
    t2 = t2.reshape(PAIR_ROWS, 128).astype(ml_dtypes.bfloat16)

    w1e = np.zeros((65, NDOM * HID), np.float32)
    w2s = np.zeros((HID, NDOM * BASE), np.float32)
    for d in range(NDOM):
        w1e[:BASE, d * HID : (d + 1) * HID] = W1[d]
        w1e[BASE, d * HID : (d + 1) * HID] = 1.0   # bias row pickup
        w2s[:, d * BASE : (d + 1) * BASE] = W2[d] * CORR
    w1e = w1e.astype(ml_dtypes.bfloat16)
    w2s = w2s.astype(ml_dtypes.bfloat16)
    idn = np.eye(128, dtype=ml_dtypes.bfloat16)
    p128 = np.hstack([w2s, idn])

    in_maps = []
    perms = []
    for c in range(NCORES):
        xc = x[c * TOK : (c + 1) * TOK].astype(np.int64)
        dc = doms[c]
        fill = list(np.where(dc < 0)[0])
        order = []
        cmask = np.zeros(M, np.float32)
        b0 = 0
        for d in range(NDOM):
            td = list(np.where(dc == d)[0])
            order += td
            cmask[b0 : b0 + len(td)] = 1.0
            pads = int(Wd[d]) - len(td)
            order += fill[:pads]
            fill = fill[pads:]
            b0 += int(Wd[d])
        order += fill
        slots = np.asarray(order, np.int64)          # slot -> core-local token
        assert len(slots) == TOK
        perms.append(slots)

        v = xc[slots]
        pid = (v >> 1).astype(np.int16)
        par = (v & 1).astype(np.uint8)

        chunks = []
        o = 0
        for ch in CHS:
            chunks.append(_wrap_idx(pid[o : o + ch]))
            o += ch
        idx = np.hstack(chunks)
        parP = par.reshape(-1, 128).T.copy()
        bias = (-30.0 * (1.0 - cmask)).astype(ml_dtypes.bfloat16)[None, :]

        in_maps.append(
            {"idx": idx, "parP": parP, "w1e": w1e, "bias": bias,
             "p128": p128, "t2": t2}
        )

    meta = {"M": M, "Wd": [int(w) for w in Wd]}
    return in_maps, perms, meta


def kernel(x, embed, W1, W2, member):
    in_maps, perms, meta = prepare(x, embed, W1, W2, member)
    key = (meta["M"], tuple(meta["Wd"]))
    if key not in _CACHED:
        _CACHED[key] = _build_program(meta)
    nc = _CACHED[key]

    res = run_bass_kernel_spmd(nc, in_maps, core_ids=list(range(NCORES)))

    M = meta["M"]
    kS = M // 128
    out = np.empty((B * S, BASE), np.float32)
    for c in range(NCORES):
        ym = np.asarray(res.results[c]["ym"], dtype=np.float32)      # [64, M]
        ynm = np.asarray(res.results[c]["ynm"], dtype=np.float32)    # [128, nmt*64]
        slots = perms[c]
        y_slots = np.empty((TOK, BASE), np.float32)
        nmt = ynm.shape[1] // 64
        ynm3 = ynm.reshape(128, nmt, 64).transpose(1, 0, 2).reshape(nmt * 128, 64)
        y_slots[kS * 128 :] = ynm3[: TOK - kS * 128]
        # member region last: straddle slots [kS*128, M) must come from ym
        y_slots[:M] = ym.T
        out[c * TOK + slots] = y_slots
    return out.reshape(B, S, BASE)
